# revision 6
# baseline (speedup 1.0000x reference)
"""Trainium2 Bass kernel for nn_ComplexFusionModule (dynamic-conv fusion).

Math (reference):
  dw = x1.reshape(B, 4, C1*H*W)                           # (32, 4, 1024)
  k_sum = einsum('bgi,goi->bo', dw, kg_w) + kg_b.sum(0)   # (32, 36864): the 600MB matmul
  kernels = k_sum.reshape(B*C2, C2, 3, 3)
  out1 = per-sample conv3x3(x2, kernels), pad 1
  cw = sigmoid(conv1x1(x1, cw_w) + cw_b)
  o1 = conv1x1(out1 + x2*cw, o1_w) + o1_b
  o2 = conv1x1(x1, o2_w) + o2_b ; o3 = conv1x1(x1, o3_w) + o3_b

Sharding: tensor-parallel over the generator OUT dim (36864 = 64 conv output
channels x 576).  Core c owns conv output channels [8c, 8c+8): it streams a
(4096, 4608) slice of the generator weight (75.5MB/core, the DMA roofline),
computes its k_sum slice for all 32 samples, PE-transposes per-(co,dydx)
blocks, runs the dynamic conv + sigmoid gating for its 8 channels, and emits
a partial o1 (o1_w[:, slice] @ fused_slice).  Host sums the 8 partials.
o2/o3 are batch-sharded (4 samples per core).  Matmuls run as float32r
(full PE rate, ~1e-4 matmul relative error).

Per-core ci rotation: x2's channels (and the matching ci axis of the weight
slice) are rolled so this core's own 8 channels sit at partitions 0..7 —
lets the gating read them from the padded x2 tile without a separate buffer.
"""

import numpy as np

import concourse.bacc as bacc
import concourse.mybir as mybir
import concourse.tile as tile
from concourse.bass_utils import run_bass_kernel_spmd

# dims
B, C1, C2, H, W, KS = 32, 4, 64, 16, 16, 3
IN = C1 * H * W            # 1024
GI = 4 * IN                # 4096 contraction
OUT = C2 * C2 * KS * KS    # 36864
NC = 8                     # cores
CO = C2 // NC              # 8 conv out-channels per core
OPC = CO * C2 * KS * KS    # 4608 per-core OUT slice
HW = H * W                 # 256
HP, WP = H + 2, W + 2      # padded 18x18
R = C2 * KS * KS           # 576 = (ci,dydx) contraction per out channel
BPC = B // NC              # 4 samples per core for o2/o3
KC = GI // 128             # 32 k-chunks

F32 = mybir.dt.float32
F32R = mybir.dt.float32r
DFR = F32R                 # dtype for derived (on-chip produced) matmul operands

SWEEPS = [[0, 1, 2], [3, 4, 5], [6, 7, 8]]  # 9 psum chunks, bank-limited split

_compiled = None
LAST_EXEC_TIME_NS = None
TRACE = False


def _build():
    nc = bacc.Bacc("TRN2", target_bir_lowering=False, debug=False, num_devices=NC)

    # per-core DRAM inputs
    w2 = nc.dram_tensor("w2", [GI, OPC], F32R, kind="ExternalInput")
    biasr = nc.dram_tensor("biasr", [1, OPC], F32R, kind="ExternalInput")
    dwt = nc.dram_tensor("dwt", [128, KC, B], F32R, kind="ExternalInput")
    x2p = nc.dram_tensor("x2p", [C2, B, HP, WP], DFR, kind="ExternalInput")
    x1p = nc.dram_tensor("x1p", [4 * C1, B, H, W], DFR, kind="ExternalInput")
    cwt = nc.dram_tensor("cwt", [4 * C1, CO], DFR, kind="ExternalInput")
    cwb = nc.dram_tensor("cwb", [CO, 1], F32, kind="ExternalInput")
    o1t = nc.dram_tensor("o1t", [CO, C2], DFR, kind="ExternalInput")
    o23t = nc.dram_tensor("o23t", [4 * C1, C1 + 3 * C1], DFR, kind="ExternalInput")
    x1o = nc.dram_tensor("x1o", [4 * C1, BPC, HW], DFR, kind="ExternalInput")
    ones = nc.dram_tensor("ones", [1, B], F32R, kind="ExternalInput")
    ident = nc.dram_tensor("ident", [B, B], DFR, kind="ExternalInput")

    # per-core DRAM outputs
    o1p = nc.dram_tensor("o1p", [B, C2, H, W], F32, kind="ExternalOutput")
    o23p = nc.dram_tensor("o23p", [C1 + 3 * C1, BPC, HW], F32, kind="ExternalOutput")

    with tile.TileContext(nc) as tc:
        with (
            tc.tile_pool(name="consts", bufs=1) as consts,
            tc.tile_pool(name="wpool", bufs=3) as wpool,
            tc.tile_pool(name="work", bufs=1) as work,
            tc.tile_pool(name="small", bufs=2) as small,
            tc.tile_pool(name="ps1", bufs=3, space="PSUM") as ps1,
            tc.tile_pool(name="pst", bufs=1, space="PSUM") as pst,
            tc.tile_pool(name="ps2", bufs=1, space="PSUM") as ps2,
        ):
            # resident small tensors
            dwt_t = consts.tile([128, KC, B], F32R)
            nc.sync.dma_start(dwt_t[:], dwt[:])
            biasr_t = consts.tile([1, OPC], F32R)
            nc.sync.dma_start(biasr_t[:], biasr[:])
            ones_t = consts.tile([1, B], F32R)
            nc.sync.dma_start(ones_t[:], ones[:])
            ident_t = consts.tile([B, B], DFR)
            nc.sync.dma_start(ident_t[:], ident[:])
            x2p_t = consts.tile([C2, B, HP, WP], DFR)
            nc.sync.dma_start(x2p_t[:], x2p[:])
            x1p_t = consts.tile([4 * C1, B, H, W], DFR)
            nc.sync.dma_start(x1p_t[:], x1p[:])
            cwt_t = consts.tile([4 * C1, CO], DFR)
            nc.sync.dma_start(cwt_t[:], cwt[:])
            cwb_t = consts.tile([CO, 1], F32)
            nc.sync.dma_start(cwb_t[:], cwb[:])
            o1t_t = consts.tile([CO, C2], DFR)
            nc.sync.dma_start(o1t_t[:], o1t[:])
            o23t_t = consts.tile([4 * C1, C1 + 3 * C1], DFR)
            nc.sync.dma_start(o23t_t[:], o23t[:])
            x1o_t = consts.tile([4 * C1, BPC, HW], DFR)
            nc.sync.dma_start(x1o_t[:], x1o[:])

            # ---- phase 1: k_sum slice = dwT.T @ W2 (+ bias) ----
            ksum = work.tile([B, OPC], DFR)   # (32, 4608), free order (co,dydx,ci)
            for sweep in SWEEPS:
                c0 = sweep[0] * 512
                ncol = (sweep[-1] + 1) * 512 - c0
                accs = {n: ps1.tile([B, 512], F32, tag="ph1", name=f"acc{n}") for n in sweep}
                for k in range(KC):
                    wt = wpool.tile([128, 3 * 512], F32R, tag="wstripe")
                    nc.sync.dma_start(wt[:, :ncol], w2[k * 128:(k + 1) * 128, c0:c0 + ncol])
                    for n in sweep:
                        nc.tensor.matmul(
                            accs[n][:],
                            dwt_t[:, k, :],
                            wt[:, n * 512 - c0:(n + 1) * 512 - c0],
                            start=(k == 0),
                            stop=False,
                        )
                for n in sweep:
                    # += bias chunk: ones^T (1,32) x bias (1,512)
                    nc.tensor.matmul(
                        accs[n][:],
                        ones_t[:],
                        biasr_t[:, n * 512:(n + 1) * 512],
                        start=False,
                        stop=True,
                    )
                    nc.vector.tensor_copy(ksum[:, n * 512:(n + 1) * 512], accs[n][:])

            # ---- phase 1.5: per-(co,dydx) PE transposes -> ksumT ----
            ksumT = work.tile([C2, KS * KS, CO, B], DFR)  # [ci, dydx, co, b]
            for co in range(CO):
                for dydx in range(KS * KS):
                    tp = pst.tile([C2, B], DFR, tag="tp")
                    nc.tensor.transpose(
                        tp[:],
                        ksum[:, co * R + dydx * C2:co * R + (dydx + 1) * C2],
                        ident_t[:],
                    )
                    nc.vector.tensor_copy(ksumT[:, dydx, co, :], tp[:])

            # ---- phase 2: per-sample conv + gating + o1 partial ----
            for b in range(B):
                cps = ps2.tile([CO, H, W], F32, tag="conv")
                for dydx in range(KS * KS):
                    dy, dx = dydx // KS, dydx % KS
                    nc.tensor.matmul(
                        cps[:],
                        ksumT[:, dydx, :, b],
                        x2p_t[:, b, dy:dy + H, dx:dx + W],
                        start=(dydx == 0),
                        stop=(dydx == KS * KS - 1),
                    )
                gps = ps2.tile([CO, H, W], F32, tag="cw")
                nc.tensor.matmul(
                    gps[:], cwt_t[:], x1p_t[:, b], start=True, stop=True,
                )
                cw_sb = small.tile([CO, H, W], F32, tag="cw_sb")
                nc.scalar.activation(
                    cw_sb[:], gps[:], mybir.ActivationFunctionType.Sigmoid,
                    bias=cwb_t[:],
                )
                gate_sb = small.tile([CO, H, W], F32, tag="gate")
                nc.vector.tensor_mul(
                    gate_sb[:], cw_sb[:], x2p_t[0:CO, b, 1:1 + H, 1:1 + W]
                )
                fused_sb = small.tile([CO, H, W], DFR, tag="fused")
                nc.vector.tensor_add(fused_sb[:], cps[:], gate_sb[:])
                ops = ps2.tile([C2, H, W], F32, tag="o1")
                nc.tensor.matmul(ops[:], o1t_t[:], fused_sb[:], start=True, stop=True)
                o1sb = small.tile([C2, H, W], F32, tag="o1sb")
                nc.vector.tensor_copy(o1sb[:], ops[:])
                nc.sync.dma_start(o1p[b], o1sb[:])

            # ---- phase 3: o2/o3 (stacked) for this core's 4 samples ----
            o23sb = work.tile([C1 + 3 * C1, BPC, HW], F32)
            for j in range(BPC):
                p23 = ps2.tile([C1 + 3 * C1, HW], F32, tag="o23")
                nc.tensor.matmul(p23[:], o23t_t[:], x1o_t[:, j], start=True, stop=True)
                nc.vector.tensor_copy(o23sb[:, j, :], p23[:])
            nc.sync.dma_start(o23p[:], o23sb[:])

    nc.compile()
    return nc


def _prep_inputs(x1, x2, kg_w, kg_b, cw_w, cw_b, o1_w, o2_w, o3_w):
    """Host-side shard/layout prep. Returns per-core input dicts."""
    # dwT in (128, kc, b) chunk-major layout
    dwt = np.ascontiguousarray(
        x1.reshape(B, GI).T.reshape(KC, 128, B).transpose(1, 0, 2)
    )
    x1p = np.ascontiguousarray(x1.reshape(B, 4 * C1, H, W).transpose(1, 0, 2, 3))

    onesv = np.ones((1, B), np.float32)
    identv = np.eye(B, dtype=np.float32)

    # generator weights viewed as (g, co, ci, ky, kx, i)
    kgw6 = kg_w.reshape(4, C2, C2, KS, KS, IN)
    bias_sum = kg_b.sum(axis=0).reshape(C2, C2, KS, KS)
    o23 = np.ascontiguousarray(np.concatenate([o2_w, o3_w], axis=0).T)  # (16, 16)

    x2pad = np.zeros((B, C2, HP, WP), np.float32)
    x2pad[:, :, 1:H + 1, 1:W + 1] = x2

    per_core = []
    for c in range(NC):
        sl = slice(c * CO, (c + 1) * CO)
        # roll ci so this core's own channels sit first (matched in x2p below)
        perm = np.roll(np.arange(C2), -c * CO)
        # W2 slice: rows (g,i), columns (co_local, ky, kx, ci[perm])
        w2c = np.ascontiguousarray(
            kgw6[:, sl][:, :, perm].transpose(0, 5, 1, 3, 4, 2).reshape(GI, OPC)
        )
        biasc = np.ascontiguousarray(
            bias_sum[sl][:, perm].transpose(0, 2, 3, 1).reshape(1, OPC)
        )
        x2pc = np.ascontiguousarray(x2pad[:, perm].transpose(1, 0, 2, 3))
        cwtc = np.ascontiguousarray(cw_w[sl, :].T)
        cwbc = np.ascontiguousarray(cw_b[sl].reshape(CO, 1))
        o1tc = np.ascontiguousarray(o1_w[:, sl].T)
        bsl = slice(c * BPC, (c + 1) * BPC)
        x1oc = np.ascontiguousarray(
            x1[bsl].reshape(BPC, 4 * C1, HW).transpose(1, 0, 2)
        )
        per_core.append({
            "w2": w2c, "biasr": biasc, "dwt": dwt, "x2p": x2pc, "x1p": x1p,
            "cwt": cwtc, "cwb": cwbc, "o1t": o1tc, "o23t": o23,
            "x1o": x1oc, "ones": onesv, "ident": identv,
        })
    return per_core


def kernel(x1, x2, kg_w, kg_b, cw_w, cw_b, o1_w, o1_b, o2_w, o2_b, o3_w, o3_b):
    global _compiled, LAST_EXEC_TIME_NS
    if _compiled is None:
        _compiled = _build()
    nc = _compiled
    in_maps = _prep_inputs(
        np.ascontiguousarray(x1, np.float32), np.ascontiguousarray(x2, np.float32),
        np.ascontiguousarray(kg_w, np.float32), np.asarray(kg_b, np.float32),
        np.asarray(cw_w, np.float32), np.asarray(cw_b, np.float32),
        np.asarray(o1_w, np.float32), np.asarray(o2_w, np.float32),
        np.asarray(o3_w, np.float32),
    )
    res = run_bass_kernel_spmd(nc, in_maps, list(range(NC)), trace=TRACE)
    LAST_EXEC_TIME_NS = res.exec_time_ns

    o1 = np.zeros((B, C2, H, W), np.float32)
    for c in range(NC):
        o1 += res.results[c]["o1p"]
    o1 += np.asarray(o1_b, np.float32)[None, :, None, None]

    o23 = np.concatenate(
        [res.results[c]["o23p"].transpose(1, 0, 2) for c in range(NC)], axis=0
    )  # (B, 16, HW)
    o2 = o23[:, :C1].reshape(B, C1, H, W) + np.asarray(o2_b, np.float32)[None, :, None, None]
    o3 = o23[:, C1:].reshape(B, 3 * C1, H, W) + np.asarray(o3_b, np.float32)[None, :, None, None]

    return (o1, np.ascontiguousarray(o2), np.ascontiguousarray(o3))


# revision 8
# speedup vs baseline: 1.1863x; 1.1863x over previous
"""Trainium2 Bass kernel for nn_ComplexFusionModule (dynamic-conv fusion).

Math (reference):
  dw = x1.reshape(B, 4, C1*H*W)                           # (32, 4, 1024)
  k_sum = einsum('bgi,goi->bo', dw, kg_w) + kg_b.sum(0)   # (32, 36864): the 600MB matmul
  kernels = k_sum.reshape(B*C2, C2, 3, 3)
  out1 = per-sample conv3x3(x2, kernels), pad 1
  cw = sigmoid(conv1x1(x1, cw_w) + cw_b)
  o1 = conv1x1(out1 + x2*cw, o1_w) + o1_b
  o2 = conv1x1(x1, o2_w) + o2_b ; o3 = conv1x1(x1, o3_w) + o3_b

Sharding: tensor-parallel over the generator OUT dim (36864 = 64 conv output
channels x 576).  Core c owns conv output channels [8c, 8c+8): it streams a
(4096, 4608) slice of the generator weight (75.5MB/core, the DMA roofline),
computes its k_sum slice for all 32 samples, PE-transposes per-(dydx,co)
blocks, runs the dynamic conv + sigmoid gating for its 8 channels, and emits
a partial o1 (o1_w[:, slice] @ fused_slice).  Host sums the 8 partials.
o2/o3 are batch-sharded (4 samples per core).  Matmuls run as float32r
(full PE rate, ~1e-4 matmul relative error).

Pipelining: the weight slice's columns are ordered (dydx, co, ci) and
streamed in 3 sweeps of 3 conv-tap groups each.  After each sweep the taps
it completes are transposed and their conv partial-products accumulate into
an SBUF out1 accumulator (seeded with the x2*sigmoid gate during sweep 0),
so conv work for sweep s overlaps sweep s+1's weight DMA.  Only the 32
small o1 matmuls remain as a tail.

Per-core ci rotation: x2's channels (and the matching ci axis of the weight
slice) are rolled so this core's own 8 channels sit at partitions 0..7 —
lets the gating read them from the padded x2 tile without a separate buffer.
"""

import numpy as np

import concourse.bacc as bacc
import concourse.mybir as mybir
import concourse.tile as tile
from concourse.bass_utils import run_bass_kernel_spmd

# dims
B, C1, C2, H, W, KS = 32, 4, 64, 16, 16, 3
IN = C1 * H * W            # 1024
GI = 4 * IN                # 4096 contraction
OUT = C2 * C2 * KS * KS    # 36864
NC = 8                     # cores
CO = C2 // NC              # 8 conv out-channels per core
OPC = CO * C2 * KS * KS    # 4608 per-core OUT slice
HW = H * W                 # 256
HP, WP = H + 2, W + 2      # padded 18x18
BPC = B // NC              # 4 samples per core for o2/o3
KC = GI // 128             # 32 k-chunks

F32 = mybir.dt.float32
F32R = mybir.dt.float32r
DFR = F32R                 # dtype for derived (on-chip produced) matmul operands

SWEEPS = [[0, 1, 2], [3, 4, 5], [6, 7, 8]]  # dydx groups / 512-col psum chunks

_compiled = None
LAST_EXEC_TIME_NS = None
TRACE = False


def _build():
    nc = bacc.Bacc("TRN2", target_bir_lowering=False, debug=False, num_devices=NC)

    # per-core DRAM inputs
    w2 = nc.dram_tensor("w2", [GI, OPC], F32R, kind="ExternalInput")
    biasr = nc.dram_tensor("biasr", [1, OPC], F32R, kind="ExternalInput")
    dwt = nc.dram_tensor("dwt", [128, KC, B], F32R, kind="ExternalInput")
    x2p = nc.dram_tensor("x2p", [C2, B, HP, WP], DFR, kind="ExternalInput")
    x1p = nc.dram_tensor("x1p", [4 * C1, B, H, W], DFR, kind="ExternalInput")
    cwt = nc.dram_tensor("cwt", [4 * C1, CO], DFR, kind="ExternalInput")
    cwb = nc.dram_tensor("cwb", [CO, 1], F32, kind="ExternalInput")
    o1t = nc.dram_tensor("o1t", [CO, C2], DFR, kind="ExternalInput")
    o23t = nc.dram_tensor("o23t", [4 * C1, C1 + 3 * C1], DFR, kind="ExternalInput")
    x1o = nc.dram_tensor("x1o", [4 * C1, BPC, HW], DFR, kind="ExternalInput")
    ones = nc.dram_tensor("ones", [1, B], F32R, kind="ExternalInput")
    ident = nc.dram_tensor("ident", [B, B], DFR, kind="ExternalInput")

    # per-core DRAM outputs
    o1p = nc.dram_tensor("o1p", [B, C2, H, W], F32, kind="ExternalOutput")
    o23p = nc.dram_tensor("o23p", [C1 + 3 * C1, BPC, HW], F32, kind="ExternalOutput")

    with tile.TileContext(nc) as tc:
        with (
            tc.tile_pool(name="consts", bufs=1) as consts,
            tc.tile_pool(name="wpool", bufs=4) as wpool,
            tc.tile_pool(name="work", bufs=1) as work,
            tc.tile_pool(name="ksp", bufs=2) as ksp,
            tc.tile_pool(name="small", bufs=2) as small,
            tc.tile_pool(name="ps1", bufs=3, space="PSUM") as ps1,
            tc.tile_pool(name="ps2", bufs=2, space="PSUM") as ps2,
        ):
            # resident small tensors
            dwt_t = consts.tile([128, KC, B], F32R)
            nc.sync.dma_start(dwt_t[:], dwt[:])
            biasr_t = consts.tile([1, OPC], F32R)
            nc.sync.dma_start(biasr_t[:], biasr[:])
            ones_t = consts.tile([1, B], F32R)
            nc.sync.dma_start(ones_t[:], ones[:])
            ident_t = consts.tile([B, B], DFR)
            nc.sync.dma_start(ident_t[:], ident[:])
            x2p_t = consts.tile([C2, B, HP, WP], DFR)
            nc.sync.dma_start(x2p_t[:], x2p[:])
            x1p_t = consts.tile([4 * C1, B, H, W], DFR)
            nc.sync.dma_start(x1p_t[:], x1p[:])
            cwt_t = consts.tile([4 * C1, CO], DFR)
            nc.sync.dma_start(cwt_t[:], cwt[:])
            cwb_t = consts.tile([CO, 1], F32)
            nc.sync.dma_start(cwb_t[:], cwb[:])
            o1t_t = consts.tile([CO, C2], DFR)
            nc.sync.dma_start(o1t_t[:], o1t[:])
            o23t_t = consts.tile([4 * C1, C1 + 3 * C1], DFR)
            nc.sync.dma_start(o23t_t[:], o23t[:])
            x1o_t = consts.tile([4 * C1, BPC, HW], DFR)
            nc.sync.dma_start(x1o_t[:], x1o[:])

            # out1 accumulator: seeded with the gate term x2*sigmoid(...),
            # conv taps accumulate on top across sweeps
            out1sb = work.tile([CO, B, H, W], DFR)
            # transposed kernels: [ci, dydx, co, b]
            ksumT = work.tile([C2, KS * KS, CO, B], DFR)

            for si, sweep in enumerate(SWEEPS):
                c0 = sweep[0] * 512
                ncol = len(sweep) * 512
                ksum_s = ksp.tile([B, 3 * 512], DFR, tag="ksum", name="ksum_s")
                accs = {n: ps1.tile([B, 512], F32, tag="ph1", name=f"acc{n}")
                        for n in sweep}
                # --- weight stream + k_sum matmuls (+ interleaved cw / o23) ---
                for k in range(KC):
                    wt = wpool.tile([128, 3 * 512], F32R, tag="wstripe")
                    nc.sync.dma_start(wt[:, :ncol], w2[k * 128:(k + 1) * 128, c0:c0 + ncol])
                    for n in sweep:
                        nc.tensor.matmul(
                            accs[n][:],
                            dwt_t[:, k, :],
                            wt[:, n * 512 - c0:(n + 1) * 512 - c0],
                            start=(k == 0),
                            stop=False,
                        )
                    if si == 0:
                        # channel gate for b=k, seeds out1sb
                        b = k
                        gps = ps2.tile([CO, H, W], F32, tag="cw")
                        nc.tensor.matmul(
                            gps[:], cwt_t[:], x1p_t[:, b], start=True, stop=True,
                        )
                        cw_sb = small.tile([CO, H, W], F32, tag="cw_sb")
                        nc.scalar.activation(
                            cw_sb[:], gps[:], mybir.ActivationFunctionType.Sigmoid,
                            bias=cwb_t[:],
                        )
                        nc.vector.tensor_mul(
                            out1sb[:, b], cw_sb[:], x2p_t[0:CO, b, 1:1 + H, 1:1 + W]
                        )
                    if si == 1 and k < BPC:
                        # o2/o3 (stacked) for this core's 4 samples
                        p23 = ps2.tile([C1 + 3 * C1, HW], F32, tag="o23", bufs=1, name="p23")
                        nc.tensor.matmul(
                            p23[:], o23t_t[:], x1o_t[:, k], start=True, stop=True,
                        )
                        o23sb = small.tile([C1 + 3 * C1, HW], F32, tag="o23sb")
                        nc.vector.tensor_copy(o23sb[:], p23[:])
                        nc.sync.dma_start(o23p[:, k], o23sb[:])
                # bias += and copy out of PSUM
                for n in sweep:
                    nc.tensor.matmul(
                        accs[n][:],
                        ones_t[:],
                        biasr_t[:, n * 512:(n + 1) * 512],
                        start=False,
                        stop=True,
                    )
                    nc.vector.tensor_copy(
                        ksum_s[:, (n - sweep[0]) * 512:(n - sweep[0] + 1) * 512],
                        accs[n][:],
                    )
                # --- transpose this sweep's (dydx, co) blocks ---
                for dydx in sweep:
                    for co in range(CO):
                        off = (dydx - sweep[0]) * 512 + co * C2
                        tp = ps2.tile([C2, B], DFR, tag="cw", name="tp")
                        nc.tensor.transpose(
                            tp[:], ksum_s[:, off:off + C2], ident_t[:]
                        )
                        nc.vector.tensor_copy(ksumT[:, dydx, co, :], tp[:])
                # --- conv partial products for this sweep's taps ---
                for b in range(B):
                    cps = ps2.tile([CO, H, W], F32, tag="conv")
                    for j, dydx in enumerate(sweep):
                        dy, dx = dydx // KS, dydx % KS
                        nc.tensor.matmul(
                            cps[:],
                            ksumT[:, dydx, :, b],
                            x2p_t[:, b, dy:dy + H, dx:dx + W],
                            start=(j == 0),
                            stop=(j == len(sweep) - 1),
                        )
                    nc.vector.tensor_add(out1sb[:, b], out1sb[:, b], cps[:])

            # ---- tail: o1 partials ----
            for b in range(B):
                ops = ps2.tile([C2, H, W], F32, tag="conv", name="ops")
                nc.tensor.matmul(ops[:], o1t_t[:], out1sb[:, b], start=True, stop=True)
                o1sb = small.tile([C2, H, W], F32, tag="o1sb")
                nc.vector.tensor_copy(o1sb[:], ops[:])
                nc.sync.dma_start(o1p[b], o1sb[:])

    nc.compile()
    return nc


def _prep_inputs(x1, x2, kg_w, kg_b, cw_w, cw_b, o1_w, o2_w, o3_w):
    """Host-side shard/layout prep. Returns per-core input dicts."""
    # dwT in (128, kc, b) chunk-major layout
    dwt = np.ascontiguousarray(
        x1.reshape(B, GI).T.reshape(KC, 128, B).transpose(1, 0, 2)
    )
    x1p = np.ascontiguousarray(x1.reshape(B, 4 * C1, H, W).transpose(1, 0, 2, 3))

    onesv = np.ones((1, B), np.float32)
    identv = np.eye(B, dtype=np.float32)

    # generator weights viewed as (g, co, ci, ky, kx, i)
    kgw6 = kg_w.reshape(4, C2, C2, KS, KS, IN)
    bias_sum = kg_b.sum(axis=0).reshape(C2, C2, KS, KS)
    o23 = np.ascontiguousarray(np.concatenate([o2_w, o3_w], axis=0).T)  # (16, 16)

    x2pad = np.zeros((B, C2, HP, WP), np.float32)
    x2pad[:, :, 1:H + 1, 1:W + 1] = x2

    per_core = []
    for c in range(NC):
        sl = slice(c * CO, (c + 1) * CO)
        # roll ci so this core's own channels sit first (matched in x2p below)
        perm = np.roll(np.arange(C2), -c * CO)
        # W2 slice: rows (g,i), columns (ky, kx, co_local, ci[perm])
        w2c = np.ascontiguousarray(
            kgw6[:, sl][:, :, perm].transpose(0, 5, 3, 4, 1, 2).reshape(GI, OPC)
        )
        biasc = np.ascontiguousarray(
            bias_sum[sl][:, perm].transpose(2, 3, 0, 1).reshape(1, OPC)
        )
        x2pc = np.ascontiguousarray(x2pad[:, perm].transpose(1, 0, 2, 3))
        cwtc = np.ascontiguousarray(cw_w[sl, :].T)
        cwbc = np.ascontiguousarray(cw_b[sl].reshape(CO, 1))
        o1tc = np.ascontiguousarray(o1_w[:, sl].T)
        bsl = slice(c * BPC, (c + 1) * BPC)
        x1oc = np.ascontiguousarray(
            x1[bsl].reshape(BPC, 4 * C1, HW).transpose(1, 0, 2)
        )
        per_core.append({
            "w2": w2c, "biasr": biasc, "dwt": dwt, "x2p": x2pc, "x1p": x1p,
            "cwt": cwtc, "cwb": cwbc, "o1t": o1tc, "o23t": o23,
            "x1o": x1oc, "ones": onesv, "ident": identv,
        })
    return per_core


def kernel(x1, x2, kg_w, kg_b, cw_w, cw_b, o1_w, o1_b, o2_w, o2_b, o3_w, o3_b):
    global _compiled, LAST_EXEC_TIME_NS
    if _compiled is None:
        _compiled = _build()
    nc = _compiled
    in_maps = _prep_inputs(
        np.ascontiguousarray(x1, np.float32), np.ascontiguousarray(x2, np.float32),
        np.ascontiguousarray(kg_w, np.float32), np.asarray(kg_b, np.float32),
        np.asarray(cw_w, np.float32), np.asarray(cw_b, np.float32),
        np.asarray(o1_w, np.float32), np.asarray(o2_w, np.float32),
        np.asarray(o3_w, np.float32),
    )
    res = run_bass_kernel_spmd(nc, in_maps, list(range(NC)), trace=TRACE)
    LAST_EXEC_TIME_NS = res.exec_time_ns

    o1 = np.zeros((B, C2, H, W), np.float32)
    for c in range(NC):
        o1 += res.results[c]["o1p"]
    o1 += np.asarray(o1_b, np.float32)[None, :, None, None]

    o23 = np.concatenate(
        [res.results[c]["o23p"].transpose(1, 0, 2) for c in range(NC)], axis=0
    )  # (B, 16, HW)
    o2 = o23[:, :C1].reshape(B, C1, H, W) + np.asarray(o2_b, np.float32)[None, :, None, None]
    o3 = o23[:, C1:].reshape(B, 3 * C1, H, W) + np.asarray(o3_b, np.float32)[None, :, None, None]

    return (o1, np.ascontiguousarray(o2), np.ascontiguousarray(o3))
